# revision 29
# baseline (speedup 1.0000x reference)
"""Trainium2 Bass kernel for nn_ClearMeshLoss.

Sharding: pred-point axis (N=8192) split 8 ways; each core computes
  - its 1024x8192 slab of the pairwise sq-dist matrix via PE matmuls (K=5 lift),
  - the slab is staged PSUM->SBUF as f16 by the ACT engine (cast + drain),
  - row minima via a tile-local f16 min-fold tree on DVE (2x perf mode),
  - exact-argmin via f16 slab spill to DRAM + indirect gather of the winning
    512-wide tile + one masked-iota sum-accumulate pass,
  - column-min accumulation as f16 tensor-min split between DVE and Pool;
    the [128, 8192] per-core accumulator is written out and the host does
    the final partition/core min + mean,
  - exact nearest-gt indices (nnidx) are returned; the host combine gathers
    the matched gt normals and evaluates the cosine term,
  - its slice of the SDF L1 sum,
  - edge-sharpness / watertight terms: host supplies only a lexsort ORDERING of
    the 120k edge keys (plus gathered per-edge face-vertex layout); the device
    verifies sortedness and computes face normals, dihedral cosines, run-length
    counts, and all sums. A sort-order violation raises at runtime.
"""
import numpy as np

import concourse.bass as bass
import concourse.mybir as mybir
import concourse.tile as tile
from concourse import bacc
from concourse.bass_utils import run_bass_kernel_spmd

P = 128
N = 8192          # pred points (total)
M = 8192          # gt points
NC_CORES = 8
NPC = N // NC_CORES          # 1024 pred rows per core
IB = NPC // P                # 8 i-blocks per core
T = 32                       # 256-wide j-tiles per row
NS = 65536
NSC = NS // NC_CORES         # 8192 sdf elems per core
V = 20000
F = 40000

CHAMFER_W, NORMAL_W, EDGE_W, WATERTIGHT_W, SDF_W = 1.0, 0.5, 0.3, 0.2, 1.0
DIHEDRAL_THRESHOLD = 0.5
EPS_COS = 1e-8
EPS_NRM = 1e-12

# edge pipeline: 3F = 120000 edges padded to 2^17, laid out [128, 1024] with a
# 3-column overlap so run/pair/cos windows never cross partitions
TE = 3 * F                 # 120000 real edges
TEP = 131072               # padded
EW = TEP // P              # 1024 own columns per partition
EWo = EW + 3               # own + 3 overlap columns (host-side full layout)
EWC = EW // NC_CORES       # 128 own columns per partition per core
EWoC = EWC + 3             # per-core slice width

# colacc: all merge blocks run on DVE (Pool/DMA cannot do elementwise min);
# block 0 is a 4x-mode DVE copy
COLACC_DVE = (1, 2, 3, 4, 5, 6, 7)

KERNEL_TRACE = False
TRACE_SINK = None
_CACHED_NC = None

f32 = mybir.dt.float32
f32r = mybir.dt.float32r
f16 = mybir.dt.float16
i32 = mybir.dt.int32
Alu = mybir.AluOpType
Ax = mybir.AxisListType
Act = mybir.ActivationFunctionType


def _build_program():
    nc = bacc.Bacc("TRN2", target_bir_lowering=False, debug=False,
                   num_devices=NC_CORES)

    # ---- I/O ----
    p5 = nc.dram_tensor("p5", [5, NPC], f32r, kind="ExternalInput")
    g5 = nc.dram_tensor("g5", [5, M], f32r, kind="ExternalInput")
    ps = nc.dram_tensor("ps", [P, NSC // P], f32, kind="ExternalInput")
    gs = nc.dram_tensor("gs", [P, NSC // P], f32, kind="ExternalInput")

    elo = nc.dram_tensor("elo", [P, EWoC], i32, kind="ExternalInput")
    ehi = nc.dram_tensor("ehi", [P, EWoC], i32, kind="ExternalInput")
    eid = nc.dram_tensor("eid", [P, EWoC], i32, kind="ExternalInput")
    vfs = nc.dram_tensor("vfs", [P, EWoC, 9], f32, kind="ExternalInput")

    rowmin_o = nc.dram_tensor("rowmin", [P, IB], f32, kind="ExternalOutput")
    epart_o = nc.dram_tensor("epart", [P, 4], f32, kind="ExternalOutput")
    nnidx_o = nc.dram_tensor("nnidx", [P, IB], i32, kind="ExternalOutput")
    colacc0_o = nc.dram_tensor("colacc0", [P, M], f16, kind="ExternalOutput")
    colacc1_o = nc.dram_tensor("colacc1", [P, M], f16, kind="ExternalOutput")
    sdfsum_o = nc.dram_tensor("sdfsum", [P, 1], f32, kind="ExternalOutput")

    # DRAM mirror of the f16 slab: row (ib, p, t) = slab[p, 512t:512(t+1)]
    mirror = nc.dram_tensor("mirror", [IB * P * T, 256], f16, kind="Internal")

    with tile.TileContext(nc) as tc:
        with (
            tc.tile_pool(name="const", bufs=1) as cpool,
            tc.tile_pool(name="swork", bufs=4) as swork,
            tc.tile_pool(name="ssm", bufs=4) as ssm,
            tc.tile_pool(name="sbig", bufs=4) as sbig,
            tc.tile_pool(name="tree", bufs=2) as trp,
            tc.tile_pool(name="rns", bufs=8) as rnp,
            tc.tile_pool(name="acc", bufs=1) as accp,
            tc.tile_pool(name="psum", bufs=2, space="PSUM") as pp,
            tc.tile_pool(name="ep", bufs=1) as ep,
        ):
            # ---- constants ----
            it256_i = cpool.tile([P, 256], i32)
            nc.gpsimd.iota(it256_i[:], [[1, 256]], channel_multiplier=0)
            iota256p1 = cpool.tile([P, 256], f32)   # 1..256
            nc.vector.tensor_copy(iota256p1[:], it256_i[:])
            nc.vector.tensor_scalar(out=iota256p1[:], in0=iota256p1[:],
                                    scalar1=1.0, scalar2=None, op0=Alu.add)

            itT8_i = cpool.tile([P, T, 8], i32)
            nc.gpsimd.iota(itT8_i[:], [[1, T], [0, 8]], channel_multiplier=0)
            iotaT8MB = cpool.tile([P, T * 8], f32)  # tile id - 64
            nc.vector.tensor_copy(iotaT8MB[:],
                                  itT8_i[:].rearrange("p t k -> p (t k)"))
            nc.vector.tensor_scalar(out=iotaT8MB[:], in0=iotaT8MB[:],
                                    scalar1=64.0, scalar2=None,
                                    op0=Alu.subtract)

            nbias = cpool.tile([P, 1], f32)      # -0.5 bias for edge relu
            nc.gpsimd.memset(nbias[:], -0.5)

            rowb_i = cpool.tile([P, 1], i32)     # p * T
            nc.gpsimd.iota(rowb_i[:], [[1, 1]], channel_multiplier=T)
            rowb_f = cpool.tile([P, 1], f32)
            nc.vector.tensor_copy(rowb_f[:], rowb_i[:])

            # ---- load lifted operands (first: the first matmul needs only
            # ---- p5 and the first g5 chunk) ----
            p5_sb = cpool.tile([5, NPC], f32r)
            nc.sync.dma_start(p5_sb[:], p5.ap())
            g5_sb = cpool.tile([5, M], f32r)
            # split the 32KB/partition load into eighths on two queues so
            # the first matmul can start after ~1/8 of the transfer
            for q in range(8):
                eng = nc.sync if q % 2 == 0 else nc.scalar
                eng.dma_start(g5_sb[:, q * 1024:(q + 1) * 1024],
                              g5.ap()[:, q * 1024:(q + 1) * 1024])

            # ---- sdf L1 partial ----
            ps_sb = ssm.tile([P, NSC // P], f32)
            gs_sb = ssm.tile([P, NSC // P], f32)
            nc.sync.dma_start(ps_sb[:], ps.ap())
            nc.sync.dma_start(gs_sb[:], gs.ap())
            sdiff = ssm.tile([P, NSC // P], f32)
            nc.vector.tensor_tensor(out=sdiff[:], in0=ps_sb[:], in1=gs_sb[:],
                                    op=Alu.subtract)
            sdabs = ssm.tile([P, NSC // P], f32)
            sdfsum = ssm.tile([P, 1], f32)
            nc.scalar.activation(sdabs[:], sdiff[:], Act.Abs,
                                 accum_out=sdfsum[:])
            nc.sync.dma_start(sdfsum_o.ap(), sdfsum[:])

            # ---- edge inputs (loaded early; compute chunks interleave with
            # ---- the chamfer i-blocks to fill DVE bubbles) ----
            elo_t = ep.tile([P, EWoC], i32)
            ehi_t = ep.tile([P, EWoC], i32)
            eid_t = ep.tile([P, EWoC], i32)
            vfs_t = ep.tile([P, EWoC, 9], f32)
            nc.sync.dma_start(elo_t[:], elo.ap())
            nc.sync.dma_start(ehi_t[:], ehi.ap())
            nc.sync.dma_start(eid_t[:], eid.ap())
            nc.sync.dma_start(vfs_t[:], vfs.ap())

            edge_state = {}

            def edge_chunk0():
                W1 = EWoC - 1
                dlo = ep.tile([P, W1], i32, tag="ti1")
                nc.vector.tensor_tensor(out=dlo[:], in0=elo_t[:, 1:],
                                        in1=elo_t[:, :-1], op=Alu.not_equal)
                dhi = ep.tile([P, W1], i32, tag="ti2")
                nc.vector.tensor_tensor(out=dhi[:], in0=ehi_t[:, 1:],
                                        in1=ehi_t[:, :-1], op=Alu.not_equal)
                rs = ep.tile([P, W1], i32, tag="rs")
                nc.vector.tensor_tensor(out=rs[:], in0=dlo[:], in1=dhi[:],
                                        op=Alu.logical_or)
                # p2 = rs0 AND NOT rs1 AND rs2; with rs in {0,1}:
                # rs0 AND NOT rs1 == rs0 > rs1
                p2 = ep.tile([P, EWC], i32, tag="p2")
                nc.vector.tensor_tensor(out=p2[:], in0=rs[:, 0:EWC],
                                        in1=rs[:, 1:EWC + 1], op=Alu.is_gt)
                nc.vector.tensor_tensor(out=p2[:], in0=p2[:], in1=rs[:, 2:EWC + 2],
                                        op=Alu.logical_and)
                totali = ep.tile([P, 1], i32, tag="s1")
                with nc.allow_low_precision(reason="exact small-int counts"):
                    nc.vector.tensor_reduce(out=totali[:], in_=rs[:, 0:EWC],
                                            axis=Ax.X, op=Alu.add)
                p2f = ep.tile([P, EWC], f32, tag="p2f")
                nc.vector.tensor_copy(p2f[:], p2[:])
                edge_state.update(p2f=p2f, totali=totali)

            def edge_chunk1():
                # sort-order verification (lex on (lo, hi))
                lt1 = ep.tile([P, EWC], i32, tag="ti1")
                nc.vector.tensor_tensor(out=lt1[:], in0=elo_t[:, 1:EWC + 1],
                                        in1=elo_t[:, 0:EWC], op=Alu.is_lt)
                eq1 = ep.tile([P, EWC], i32, tag="ti3")
                nc.vector.tensor_tensor(out=eq1[:], in0=elo_t[:, 1:EWC + 1],
                                        in1=elo_t[:, 0:EWC], op=Alu.is_equal)
                lt2 = ep.tile([P, EWC], i32, tag="ti2")
                nc.vector.tensor_tensor(out=lt2[:], in0=ehi_t[:, 1:EWC + 1],
                                        in1=ehi_t[:, 0:EWC], op=Alu.is_lt)
                nc.vector.tensor_tensor(out=eq1[:], in0=eq1[:], in1=lt2[:],
                                        op=Alu.logical_and)
                nc.vector.tensor_tensor(out=eq1[:], in0=eq1[:], in1=lt1[:],
                                        op=Alu.logical_or)
                violi = ep.tile([P, 1], i32, tag="s2")
                with nc.allow_low_precision(reason="exact small-int counts"):
                    nc.vector.tensor_reduce(out=violi[:], in_=eq1[:], axis=Ax.X,
                                            op=Alu.add)
                edge_state.update(violi=violi)

            def edge_chunk2():
                # face id = rint((eid-1)/3); same-face pair detection
                eidf = ep.tile([P, EWoC], f32, tag="tf1")
                nc.vector.tensor_copy(eidf[:], eid_t[:])
                nc.vector.tensor_scalar(out=eidf[:], in0=eidf[:], scalar1=-1.0,
                                        scalar2=0.33333334, op0=Alu.add,
                                        op1=Alu.mult)
                fidi = ep.tile([P, EWoC], i32, tag="ti4")
                nc.vector.tensor_copy(fidi[:], eidf[:])
                samef = ep.tile([P, EWC], i32, tag="ti1")
                nc.vector.tensor_tensor(out=samef[:], in0=fidi[:, 1:EWC + 1],
                                        in1=fidi[:, 2:EWC + 2], op=Alu.is_equal)
                samef_f = ep.tile([P, EWC], f32, tag="tf2")
                nc.vector.tensor_copy(samef_f[:], samef[:])
                # XLA-FMA artifact emulation: degenerate face with v1==v2 gets a
                # unit normal in the reference, so a self-paired edge scores 0.5
                eqv = ep.tile([P, EWoC, 3], f32, tag="e1")
                nc.vector.tensor_tensor(out=eqv[:], in0=vfs_t[:, :, 3:6],
                                        in1=vfs_t[:, :, 6:9], op=Alu.is_equal)
                alleq = ep.tile([P, EWoC], f32, tag="tf3")
                nc.vector.tensor_reduce(out=alleq[:], in_=eqv[:], axis=Ax.X,
                                        op=Alu.min)
                ovr = ep.tile([P, EWC], f32, tag="tf4")
                nc.vector.tensor_tensor(out=ovr[:], in0=samef_f[:],
                                        in1=alleq[:, 1:EWC + 1], op=Alu.mult)
                edge_state.update(ovr=ovr)

            def edge_chunk3():
                # face normal cross products (e1 x e2)
                e1t = ep.tile([P, EWoC, 3], f32, tag="e1")
                nc.vector.tensor_tensor(out=e1t[:], in0=vfs_t[:, :, 3:6],
                                        in1=vfs_t[:, :, 0:3], op=Alu.subtract)
                e2t = ep.tile([P, EWoC, 3], f32, tag="e2")
                nc.vector.tensor_tensor(out=e2t[:], in0=vfs_t[:, :, 6:9],
                                        in1=vfs_t[:, :, 0:3], op=Alu.subtract)
                n3 = ep.tile([P, EWoC, 3], f32, tag="n3")
                for k in range(3):
                    ka, kb = (k + 1) % 3, (k + 2) % 3
                    m1 = ep.tile([P, EWoC], f32, tag="tm1")
                    m2 = ep.tile([P, EWoC], f32, tag="tm2")
                    nc.vector.tensor_tensor(out=m1[:], in0=e1t[:, :, ka],
                                            in1=e2t[:, :, kb], op=Alu.mult)
                    nc.vector.tensor_tensor(out=m2[:], in0=e1t[:, :, kb],
                                            in1=e2t[:, :, ka], op=Alu.mult)
                    nc.vector.tensor_tensor(out=n3[:, :, k], in0=m1[:], in1=m2[:],
                                            op=Alu.subtract)
                edge_state.update(n3=n3)

            def edge_chunk4():
                n3 = edge_state["n3"]
                nsq = ep.tile([P, EWoC], f32, tag="tm3")
                nc.vector.tensor_tensor(out=nsq[:], in0=n3[:, :, 0],
                                        in1=n3[:, :, 0], op=Alu.mult)
                for k in (1, 2):
                    mk = ep.tile([P, EWoC], f32, tag="tm1")
                    nc.vector.tensor_tensor(out=mk[:], in0=n3[:, :, k],
                                            in1=n3[:, :, k], op=Alu.mult)
                    nc.vector.tensor_tensor(out=nsq[:], in0=nsq[:], in1=mk[:],
                                            op=Alu.add)
                nc.scalar.activation(nsq[:], nsq[:], Act.Sqrt)
                nc.vector.tensor_scalar(out=nsq[:], in0=nsq[:], scalar1=EPS_NRM,
                                        scalar2=None, op0=Alu.max)
                nc.vector.reciprocal(nsq[:], nsq[:])
                for k in range(3):
                    nc.vector.tensor_tensor(out=n3[:, :, k], in0=n3[:, :, k],
                                            in1=nsq[:], op=Alu.mult)

            def edge_chunk5():
                n3 = edge_state["n3"]
                p2f = edge_state["p2f"]
                ovr = edge_state["ovr"]
                totali = edge_state["totali"]
                violi = edge_state["violi"]
                prod = ep.tile([P, EWC, 3], f32, tag="e1")
                nc.vector.tensor_tensor(out=prod[:], in0=n3[:, 1:EWC + 1, :],
                                        in1=n3[:, 2:EWC + 2, :], op=Alu.mult)
                cosa = ep.tile([P, EWC], f32, tag="tf1")
                nc.vector.tensor_reduce(out=cosa[:], in_=prod[:], axis=Ax.X,
                                        op=Alu.add)
                nc.scalar.activation(cosa[:], cosa[:], Act.Relu, bias=nbias[:, :1])
                d5 = ep.tile([P, EWC], f32, tag="tf3")
                nc.scalar.activation(d5[:], cosa[:], Act.Copy, bias=0.5,
                                     scale=-1.0)
                nc.vector.tensor_tensor(out=d5[:], in0=d5[:], in1=ovr[:],
                                        op=Alu.mult)
                nc.vector.tensor_tensor(out=cosa[:], in0=cosa[:], in1=d5[:],
                                        op=Alu.add)
                nc.vector.tensor_tensor(out=cosa[:], in0=cosa[:], in1=p2f[:],
                                        op=Alu.mult)
                spart = ep.tile([P, 1], f32, tag="s3")
                nc.vector.tensor_reduce(out=spart[:], in_=cosa[:], axis=Ax.X,
                                        op=Alu.add)
                cnt2p = ep.tile([P, 1], f32, tag="s4")
                nc.vector.tensor_reduce(out=cnt2p[:], in_=p2f[:], axis=Ax.X,
                                        op=Alu.add)
                epk = ep.tile([P, 4], f32, tag="s5")
                nc.vector.tensor_copy(epk[:, 0:1], totali[:])
                nc.vector.tensor_copy(epk[:, 1:2], cnt2p[:])
                nc.vector.tensor_copy(epk[:, 2:3], spart[:])
                nc.vector.tensor_copy(epk[:, 3:4], violi[:])
                nc.sync.dma_start(epart_o.ap(), epk[:])

            edge_chunks = [edge_chunk0, edge_chunk1, edge_chunk2, edge_chunk3,
                           edge_chunk4, edge_chunk5]

            # ---- chamfer ----
            colacc0 = accp.tile([P, M], f16)
            colacc1 = accp.tile([P, M], f16)
            nnidx_i = cpool.tile([P, IB], i32)
            rowmin_all = cpool.tile([P, IB], f32)

            for ib in range(IB):
                slab = sbig.tile([P, M], f16, tag="slab")
                for jq in range(4):
                    psq = pp.tile([P, 2048], f32, tag="psq")
                    for k in range(4):
                        j0 = (jq * 4 + k) * 512
                        nc.tensor.matmul(psq[:, k * 512:(k + 1) * 512],
                                         lhsT=p5_sb[:, ib * P:(ib + 1) * P],
                                         rhs=g5_sb[:, j0:j0 + 512],
                                         start=True, stop=True)
                    # drain PSUM quarter -> f16 slab (ACT engine, casts)
                    nc.scalar.activation(slab[:, jq * 2048:(jq + 1) * 2048],
                                         psq[:], Act.Copy)
                    # spill this quarter for the winning-tile gather; the
                    # gather only waits ~one quarter-transfer after staging
                    nc.sync.dma_start(
                        mirror.ap()[ib * P * T:(ib + 1) * P * T, :]
                        .rearrange("(p t) k -> p t k", p=P)[:, 8 * jq:8 * jq + 8],
                        slab[:, jq * 2048:(jq + 1) * 2048]
                        .rearrange("p (t k) -> p t k", t=8))

                # tile-local min-fold tree: [P, 16, 512] -> [P, 16, 8]
                s3 = slab[:].rearrange("p (t k) -> p t k", t=T)
                lv = s3
                w = 256
                for li in range(5):
                    nxt = trp.tile([P, T, w // 2], f16, tag=f"L{li}")
                    nc.vector.tensor_tensor(out=nxt[:], in0=lv[:, :, 0:w // 2],
                                            in1=lv[:, :, w // 2:w], op=Alu.min)
                    lv = nxt
                    w //= 2
                rn = rnp.tile([P, 1], f16, tag="rn")
                nc.vector.tensor_reduce(out=rn[:], in_=lv[:], axis=Ax.XY,
                                        op=Alu.min)
                nc.scalar.activation(rowmin_all[:, ib:ib + 1], rn[:], Act.Copy)

                # winning tile t* = first tile whose L6 entry equals rn
                # (cand entries: t*-64 at matches, 0 elsewhere; min = t*-64)
                cand16 = swork.tile([P, T * 8], f32, tag="cand16")
                nc.vector.scalar_tensor_tensor(
                    out=cand16[:], in0=lv[:].rearrange("p t k -> p (t k)"),
                    scalar=rn[:, :1], in1=iotaT8MB[:],
                    op0=Alu.is_equal, op1=Alu.mult)
                argtm = swork.tile([P, 1], f32, tag="argtm")  # t* - 64
                nc.vector.tensor_reduce(out=argtm[:], in_=cand16[:], axis=Ax.X,
                                        op=Alu.min)
                # mirror row = ib*P*T + p*T + (argtm + 64)
                ridx_f = swork.tile([P, 1], f32, tag="ridx_f")
                nc.vector.scalar_tensor_tensor(out=ridx_f[:], in0=argtm[:],
                                               scalar=float(ib * P * T + 64),
                                               in1=rowb_f[:], op0=Alu.add,
                                               op1=Alu.add)
                ridx_i = swork.tile([P, 1], i32, tag="ridx_i")
                nc.vector.tensor_copy(ridx_i[:], ridx_f[:])
                win = swork.tile([P, 256], f16, tag="win")
                nc.gpsimd.indirect_dma_start(
                    out=win[:], out_offset=None, in_=mirror.ap(),
                    in_offset=bass.IndirectOffsetOnAxis(ap=ridx_i[:, :1], axis=0))

                # column-min accumulate (f16, DVE 2x; copies are 4x);
                # two accumulators: colacc0 streams out mid-loop on the ACT
                # queue, colacc1's final merge is split so its DMA overlaps
                acc = colacc0 if ib < 4 else colacc1
                if ib % 4 == 0:
                    nc.vector.tensor_copy(acc[:], slab[:])
                elif ib == 3:
                    for h in range(2):
                        sl = slice(h * (M // 2), (h + 1) * (M // 2))
                        nc.vector.tensor_tensor(out=acc[:, sl], in0=acc[:, sl],
                                                in1=slab[:, sl], op=Alu.min)
                        nc.sync.dma_start(colacc0_o.ap()[:, sl], acc[:, sl])
                elif ib == IB - 1:
                    for h in range(2):
                        sl = slice(h * (M // 2), (h + 1) * (M // 2))
                        nc.vector.tensor_tensor(out=acc[:, sl], in0=acc[:, sl],
                                                in1=slab[:, sl], op=Alu.min)
                        nc.sync.dma_start(colacc1_o.ap()[:, sl], acc[:, sl])
                else:
                    nc.vector.tensor_tensor(out=acc[:], in0=acc[:],
                                            in1=slab[:], op=Alu.min)

                for ci, slot in ((0, 0), (1, 0), (2, 1), (3, 5), (4, 6),
                                 (5, 7)):
                    if slot == ib:
                        edge_chunks[ci]()
                # u*+1 = sum over the window of (win == rn) * (iota+1)
                cand = swork.tile([P, 256], f32, tag="cand")
                us = swork.tile([P, 1], f32, tag="us")
                nc.vector.scalar_tensor_tensor(out=cand[:], in0=win[:],
                                               scalar=rn[:, :1],
                                               in1=iota256p1[:],
                                               op0=Alu.is_equal, op1=Alu.mult,
                                               accum_out=us[:])
                # j+1-16384 = (t*-64)*256 + (u*+1); j -> int (host clips)
                jp1 = swork.tile([P, 1], f32, tag="jp1")
                nc.vector.scalar_tensor_tensor(out=jp1[:], in0=argtm[:],
                                               scalar=256.0, in1=us[:],
                                               op0=Alu.mult, op1=Alu.add)
                nc.vector.tensor_scalar(out=nnidx_i[:, ib:ib + 1],
                                        in0=jp1[:], scalar1=16383.0,
                                        scalar2=None, op0=Alu.add)

            nc.sync.dma_start(nnidx_o.ap(), nnidx_i[:])
            nc.sync.dma_start(rowmin_o.ap(), rowmin_all[:])

    nc.compile()
    return nc


def _edge_host_inputs(verts, faces):
    """Host provides ORDERING + gathered layout only (lexsort + indexing);
    the device verifies sortedness and does all the arithmetic."""
    a = faces.reshape(-1).astype(np.int32)
    b = np.roll(faces, -1, axis=1).reshape(-1).astype(np.int32)
    lo = np.minimum(a, b)
    hi = np.maximum(a, b)
    perm = np.lexsort((hi, lo)).astype(np.int32)   # stable key order

    loS = np.full(TEP, 20001, np.int64)
    hiS = np.zeros(TEP, np.int64)
    eidS = np.zeros(TEP, np.int32)
    loS[:TE] = lo[perm]
    hiS[:TE] = hi[perm]
    eidS[:TE] = perm
    loS = loS.astype(np.int32)
    hiS = hiS.astype(np.int32)
    vfS = np.zeros((TEP, 9), np.float32)
    vfS[:TE] = verts[faces[perm // 3]].reshape(TE, 9)

    def overlap(arr, lo_sent, hi_sent):
        out = np.empty((P, EWo) + arr.shape[1:], arr.dtype)
        for c in range(EWo):
            i = np.arange(P) * EW + c - 1
            valid = (i >= 0) & (i < TEP)
            out[valid, c] = arr[i[valid]]
            out[~valid, c] = lo_sent if (c == 0) else hi_sent
        return out

    return {
        "elo": overlap(loS, -1, -2),
        "ehi": overlap(hiS, -1, -2),
        "eid": overlap(eidS, 0, 0),
        "vfs": overlap(vfS, 0.0, 0.0),
    }


def _lift_p(pts):
    """[K,3] -> [5,K] rows (x, y, z, |p|^2, 1)."""
    k = pts.shape[0]
    out = np.empty((5, k), np.float32)
    out[0:3] = pts.T
    out[3] = (pts * pts).sum(-1)
    out[4] = 1.0
    return out


def _lift_g(pts):
    """[M,3] -> [5,M] rows (-2x, -2y, -2z, 1, |g|^2)."""
    m = pts.shape[0]
    out = np.empty((5, m), np.float32)
    out[0:3] = -2.0 * pts.T
    out[3] = 1.0
    out[4] = (pts * pts).sum(-1)
    return out


def kernel(pred_sdf, gt_sdf, extracted_vertices, extracted_faces, gt_vertices,
           gt_faces, pred_points, gt_points, pred_normals, gt_normals):
    global _CACHED_NC
    if _CACHED_NC is None:
        _CACHED_NC = _build_program()
    nc = _CACHED_NC

    pp_full = np.asarray(pred_points, np.float32)[0]     # [N,3]
    gp_full = np.asarray(gt_points, np.float32)[0]       # [M,3]
    pn_full = np.asarray(pred_normals, np.float32)[0]
    gn_full = np.asarray(gt_normals, np.float32)[0]
    ps_full = np.asarray(pred_sdf, np.float32).reshape(-1)
    gs_full = np.asarray(gt_sdf, np.float32).reshape(-1)

    g5 = _lift_g(gp_full)
    edge_in = _edge_host_inputs(np.asarray(extracted_vertices, np.float32),
                                np.asarray(extracted_faces))
    in_maps = []
    for c in range(NC_CORES):
        rows = pp_full[c * NPC:(c + 1) * NPC]
        # column order (ib, p): column ib*128+p <-> core row p*8+ib
        p5c = _lift_p(rows)                               # [5, NPC] core-row order
        p5c = p5c.reshape(5, P, IB).transpose(0, 2, 1).reshape(5, NPC).copy()
        in_maps.append({
            "p5": p5c,
            "g5": g5,
            "ps": ps_full[c * NSC:(c + 1) * NSC].reshape(P, NSC // P).copy(),
            "gs": gs_full[c * NSC:(c + 1) * NSC].reshape(P, NSC // P).copy(),
            # per-core column shard of the sorted edge layout
            **{k: np.ascontiguousarray(v[:, c * EWC:c * EWC + EWoC])
               for k, v in edge_in.items()},
        })

    res = run_bass_kernel_spmd(nc, in_maps, core_ids=list(range(NC_CORES)),
                               trace=KERNEL_TRACE)
    if KERNEL_TRACE and res.exec_time_ns is not None:
        print(f"HW exec time: {res.exec_time_ns} ns")
    if TRACE_SINK is not None and res.instructions_and_trace is not None:
        TRACE_SINK["insts"] = res.instructions_and_trace[0]

    # ---- host combine ----
    rowmin_sum = 0.0
    sdf_sum = 0.0
    colmin = np.full(M, np.inf, np.float64)
    nn_global = np.empty(N, np.int64)
    for c in range(NC_CORES):
        r = res.results[c]
        rowmin_sum += r["rowmin"].astype(np.float64).sum()
        sdf_sum += r["sdfsum"].astype(np.float64).sum()
        # device row (p, ib) = pred point c*NPC + p*IB + ib
        nn_global[c * NPC:(c + 1) * NPC] = (
            r["nnidx"].astype(np.int64).reshape(-1))
        for key in ("colacc0", "colacc1"):
            ca = np.asarray(r[key])
            if ca.dtype != np.float16:
                ca = ca.view(np.float16) if ca.dtype.itemsize == 2 else ca
            colmin = np.minimum(colmin, ca.astype(np.float64).min(axis=0))

    sdf_l = SDF_W * sdf_sum / NS
    min_p2g = rowmin_sum / N
    min_g2p = colmin.mean()
    chamfer_l = CHAMFER_W * (min_p2g + min_g2p)
    # normal consistency from device nnidx (cosine math on host)
    matched = gn_full[np.clip(nn_global, 0, M - 1)]
    pnn = np.maximum(np.linalg.norm(pn_full, axis=-1), EPS_COS)
    gnn = np.maximum(np.linalg.norm(matched, axis=-1), EPS_COS)
    cosv = (pn_full * matched).sum(-1) / (pnn * gnn)
    normal_l = NORMAL_W * (1.0 - np.abs(cosv)).mean()

    ep = sum(res.results[c]["epart"].astype(np.float64)
             for c in range(NC_CORES))
    viol = ep[:, 3].sum()
    if viol != 0:
        raise RuntimeError(f"device sort-order verification failed: {viol}")
    total = ep[:, 0].sum() - 1.0      # minus the padding run
    cnt2 = ep[:, 1].sum()
    s2 = ep[:, 2].sum()
    edge = s2 / max(cnt2, 1.0) if cnt2 > 0 else 0.0
    bad = total - cnt2
    wt = bad / max(total, 1.0) if total > 0 else 0.0
    edge_l = EDGE_W * float(edge)
    wt_l = WATERTIGHT_W * float(wt)

    total = sdf_l + chamfer_l + normal_l + edge_l + wt_l
    return (np.float32(sdf_l), np.float32(chamfer_l), np.float32(normal_l),
            np.float32(edge_l), np.float32(wt_l), np.float32(total))


# revision 30
# speedup vs baseline: 1.0167x; 1.0167x over previous
"""Trainium2 Bass kernel for nn_ClearMeshLoss.

Sharding: pred-point axis (N=8192) split 8 ways; each core computes
  - its 1024x8192 slab of the pairwise sq-dist matrix via PE matmuls (K=5 lift),
  - the slab is staged PSUM->SBUF as f16 by the ACT engine (cast + drain),
  - row minima via a tile-local f16 min-fold tree on DVE (2x perf mode),
  - exact-argmin via f16 slab spill to DRAM + indirect gather of the winning
    512-wide tile + one masked-iota sum-accumulate pass,
  - column-min accumulation as f16 tensor-min split between DVE and Pool;
    the [128, 8192] per-core accumulator is written out and the host does
    the final partition/core min + mean,
  - exact nearest-gt indices (nnidx) are returned; the host combine gathers
    the matched gt normals and evaluates the cosine term,
  - its slice of the SDF L1 sum,
  - edge-sharpness / watertight terms: host supplies only a lexsort ORDERING of
    the 120k edge keys (plus gathered per-edge face-vertex layout); the device
    verifies sortedness and computes face normals, dihedral cosines, run-length
    counts, and all sums. A sort-order violation raises at runtime.
"""
import numpy as np

import concourse.bass as bass
import concourse.mybir as mybir
import concourse.tile as tile
from concourse import bacc
from concourse.bass_utils import run_bass_kernel_spmd

P = 128
N = 8192          # pred points (total)
M = 8192          # gt points
NC_CORES = 8
NPC = N // NC_CORES          # 1024 pred rows per core
IB = NPC // P                # 8 i-blocks per core
T = 32                       # 256-wide j-tiles per row
NS = 65536
NSC = NS // NC_CORES         # 8192 sdf elems per core
V = 20000
F = 40000

CHAMFER_W, NORMAL_W, EDGE_W, WATERTIGHT_W, SDF_W = 1.0, 0.5, 0.3, 0.2, 1.0
DIHEDRAL_THRESHOLD = 0.5
EPS_COS = 1e-8
EPS_NRM = 1e-12

# edge pipeline: 3F = 120000 edges padded to 2^17, laid out [128, 1024] with a
# 3-column overlap so run/pair/cos windows never cross partitions
TE = 3 * F                 # 120000 real edges
TEP = 131072               # padded
EW = TEP // P              # 1024 own columns per partition
EWo = EW + 3               # own + 3 overlap columns (host-side full layout)
EWC = EW // NC_CORES       # 128 own columns per partition per core
EWoC = EWC + 3             # per-core slice width

# colacc: all merge blocks run on DVE (Pool/DMA cannot do elementwise min);
# block 0 is a 4x-mode DVE copy
COLACC_DVE = (1, 2, 3, 4, 5, 6, 7)

KERNEL_TRACE = False
TRACE_SINK = None
_CACHED_NC = None

f32 = mybir.dt.float32
f32r = mybir.dt.float32r
f16 = mybir.dt.float16
i32 = mybir.dt.int32
Alu = mybir.AluOpType
Ax = mybir.AxisListType
Act = mybir.ActivationFunctionType


def _build_program():
    nc = bacc.Bacc("TRN2", target_bir_lowering=False, debug=False,
                   num_devices=NC_CORES)

    # ---- I/O ----
    p5 = nc.dram_tensor("p5", [5, NPC], f32r, kind="ExternalInput")
    g5 = nc.dram_tensor("g5", [5, M], f32r, kind="ExternalInput")
    ps = nc.dram_tensor("ps", [P, NSC // P], f32, kind="ExternalInput")
    gs = nc.dram_tensor("gs", [P, NSC // P], f32, kind="ExternalInput")

    elo = nc.dram_tensor("elo", [P, EWoC], i32, kind="ExternalInput")
    ehi = nc.dram_tensor("ehi", [P, EWoC], i32, kind="ExternalInput")
    eid = nc.dram_tensor("eid", [P, EWoC], i32, kind="ExternalInput")
    vfs = nc.dram_tensor("vfs", [P, EWoC, 9], f32, kind="ExternalInput")

    rowmin_o = nc.dram_tensor("rowmin", [P, IB], f32, kind="ExternalOutput")
    epart_o = nc.dram_tensor("epart", [P, 4], f32, kind="ExternalOutput")
    nnidx_o = nc.dram_tensor("nnidx", [P, IB], i32, kind="ExternalOutput")
    colacc0_o = nc.dram_tensor("colacc0", [P, M], f16, kind="ExternalOutput")
    colacc1_o = nc.dram_tensor("colacc1", [P, M], f16, kind="ExternalOutput")
    sdfsum_o = nc.dram_tensor("sdfsum", [P, 1], f32, kind="ExternalOutput")

    # DRAM mirror of the f16 slab: row (ib, p, t) = slab[p, 512t:512(t+1)]
    mirror = nc.dram_tensor("mirror", [IB * P * T, 256], f16, kind="Internal")

    with tile.TileContext(nc) as tc:
        with (
            tc.tile_pool(name="const", bufs=1) as cpool,
            tc.tile_pool(name="swork", bufs=4) as swork,
            tc.tile_pool(name="ssm", bufs=4) as ssm,
            tc.tile_pool(name="sbig", bufs=4) as sbig,
            tc.tile_pool(name="tree", bufs=2) as trp,
            tc.tile_pool(name="rns", bufs=8) as rnp,
            tc.tile_pool(name="acc", bufs=1) as accp,
            tc.tile_pool(name="psum", bufs=2, space="PSUM") as pp,
            tc.tile_pool(name="ep", bufs=1) as ep,
        ):
            # ---- constants ----
            it256_i = cpool.tile([P, 256], i32)
            nc.gpsimd.iota(it256_i[:], [[1, 256]], channel_multiplier=0)
            iota256p1 = cpool.tile([P, 256], f32)   # 1..256
            nc.vector.tensor_copy(iota256p1[:], it256_i[:])
            nc.vector.tensor_scalar(out=iota256p1[:], in0=iota256p1[:],
                                    scalar1=1.0, scalar2=None, op0=Alu.add)

            itT8_i = cpool.tile([P, T, 8], i32)
            nc.gpsimd.iota(itT8_i[:], [[1, T], [0, 8]], channel_multiplier=0)
            iotaT8MB = cpool.tile([P, T * 8], f32)  # tile id - 64
            nc.vector.tensor_copy(iotaT8MB[:],
                                  itT8_i[:].rearrange("p t k -> p (t k)"))
            nc.vector.tensor_scalar(out=iotaT8MB[:], in0=iotaT8MB[:],
                                    scalar1=64.0, scalar2=None,
                                    op0=Alu.subtract)

            nbias = cpool.tile([P, 1], f32)      # -0.5 bias for edge relu
            nc.gpsimd.memset(nbias[:], -0.5)

            rowb_i = cpool.tile([P, 1], i32)     # p * T
            nc.gpsimd.iota(rowb_i[:], [[1, 1]], channel_multiplier=T)
            rowb_f = cpool.tile([P, 1], f32)
            nc.vector.tensor_copy(rowb_f[:], rowb_i[:])

            # ---- load lifted operands (first: the first matmul needs only
            # ---- p5 and the first g5 chunk) ----
            p5_sb = cpool.tile([5, NPC], f32r)
            nc.sync.dma_start(p5_sb[:], p5.ap())
            g5_sb = cpool.tile([5, M], f32r)
            # split the 32KB/partition load into eighths on two queues so
            # the first matmul can start after ~1/8 of the transfer
            for q in range(8):
                eng = nc.sync if q % 2 == 0 else nc.scalar
                eng.dma_start(g5_sb[:, q * 1024:(q + 1) * 1024],
                              g5.ap()[:, q * 1024:(q + 1) * 1024])

            # ---- sdf L1 partial ----
            ps_sb = ssm.tile([P, NSC // P], f32)
            gs_sb = ssm.tile([P, NSC // P], f32)
            nc.sync.dma_start(ps_sb[:], ps.ap())
            nc.sync.dma_start(gs_sb[:], gs.ap())
            sdiff = ssm.tile([P, NSC // P], f32)
            nc.vector.tensor_tensor(out=sdiff[:], in0=ps_sb[:], in1=gs_sb[:],
                                    op=Alu.subtract)
            sdabs = ssm.tile([P, NSC // P], f32)
            sdfsum = ssm.tile([P, 1], f32)
            nc.scalar.activation(sdabs[:], sdiff[:], Act.Abs,
                                 accum_out=sdfsum[:])
            nc.sync.dma_start(sdfsum_o.ap(), sdfsum[:])

            # ---- edge inputs (loaded early; compute chunks interleave with
            # ---- the chamfer i-blocks to fill DVE bubbles) ----
            elo_t = ep.tile([P, EWoC], i32)
            ehi_t = ep.tile([P, EWoC], i32)
            eid_t = ep.tile([P, EWoC], i32)
            vfs_t = ep.tile([P, EWoC, 9], f32)
            nc.sync.dma_start(elo_t[:], elo.ap())
            nc.sync.dma_start(ehi_t[:], ehi.ap())
            nc.sync.dma_start(eid_t[:], eid.ap())
            nc.sync.dma_start(vfs_t[:], vfs.ap())

            edge_state = {}

            def edge_chunk0():
                W1 = EWoC - 1
                dlo = ep.tile([P, W1], i32, tag="ti1")
                nc.vector.tensor_tensor(out=dlo[:], in0=elo_t[:, 1:],
                                        in1=elo_t[:, :-1], op=Alu.not_equal)
                dhi = ep.tile([P, W1], i32, tag="ti2")
                nc.vector.tensor_tensor(out=dhi[:], in0=ehi_t[:, 1:],
                                        in1=ehi_t[:, :-1], op=Alu.not_equal)
                rs = ep.tile([P, W1], i32, tag="rs")
                nc.vector.tensor_tensor(out=rs[:], in0=dlo[:], in1=dhi[:],
                                        op=Alu.logical_or)
                # p2 = rs0 AND NOT rs1 AND rs2; with rs in {0,1}:
                # rs0 AND NOT rs1 == rs0 > rs1
                p2 = ep.tile([P, EWC], i32, tag="p2")
                nc.vector.tensor_tensor(out=p2[:], in0=rs[:, 0:EWC],
                                        in1=rs[:, 1:EWC + 1], op=Alu.is_gt)
                nc.vector.tensor_tensor(out=p2[:], in0=p2[:], in1=rs[:, 2:EWC + 2],
                                        op=Alu.logical_and)
                totali = ep.tile([P, 1], i32, tag="s1")
                with nc.allow_low_precision(reason="exact small-int counts"):
                    nc.vector.tensor_reduce(out=totali[:], in_=rs[:, 0:EWC],
                                            axis=Ax.X, op=Alu.add)
                p2f = ep.tile([P, EWC], f32, tag="p2f")
                nc.vector.tensor_copy(p2f[:], p2[:])
                edge_state.update(p2f=p2f, totali=totali)

            def edge_chunk1():
                # sort-order verification (lex on (lo, hi))
                lt1 = ep.tile([P, EWC], i32, tag="ti1")
                nc.vector.tensor_tensor(out=lt1[:], in0=elo_t[:, 1:EWC + 1],
                                        in1=elo_t[:, 0:EWC], op=Alu.is_lt)
                eq1 = ep.tile([P, EWC], i32, tag="ti3")
                nc.vector.tensor_tensor(out=eq1[:], in0=elo_t[:, 1:EWC + 1],
                                        in1=elo_t[:, 0:EWC], op=Alu.is_equal)
                lt2 = ep.tile([P, EWC], i32, tag="ti2")
                nc.vector.tensor_tensor(out=lt2[:], in0=ehi_t[:, 1:EWC + 1],
                                        in1=ehi_t[:, 0:EWC], op=Alu.is_lt)
                nc.vector.tensor_tensor(out=eq1[:], in0=eq1[:], in1=lt2[:],
                                        op=Alu.logical_and)
                nc.vector.tensor_tensor(out=eq1[:], in0=eq1[:], in1=lt1[:],
                                        op=Alu.logical_or)
                violi = ep.tile([P, 1], i32, tag="s2")
                with nc.allow_low_precision(reason="exact small-int counts"):
                    nc.vector.tensor_reduce(out=violi[:], in_=eq1[:], axis=Ax.X,
                                            op=Alu.add)
                edge_state.update(violi=violi)

            def edge_chunk2():
                # face id = rint((eid-1)/3); same-face pair detection
                eidf = ep.tile([P, EWoC], f32, tag="tf1")
                nc.vector.tensor_copy(eidf[:], eid_t[:])
                nc.vector.tensor_scalar(out=eidf[:], in0=eidf[:], scalar1=-1.0,
                                        scalar2=0.33333334, op0=Alu.add,
                                        op1=Alu.mult)
                fidi = ep.tile([P, EWoC], i32, tag="ti4")
                nc.vector.tensor_copy(fidi[:], eidf[:])
                samef = ep.tile([P, EWC], i32, tag="ti1")
                nc.vector.tensor_tensor(out=samef[:], in0=fidi[:, 1:EWC + 1],
                                        in1=fidi[:, 2:EWC + 2], op=Alu.is_equal)
                samef_f = ep.tile([P, EWC], f32, tag="tf2")
                nc.vector.tensor_copy(samef_f[:], samef[:])
                # XLA-FMA artifact emulation: degenerate face with v1==v2 gets a
                # unit normal in the reference, so a self-paired edge scores 0.5
                eqv = ep.tile([P, EWoC, 3], f32, tag="e1")
                nc.vector.tensor_tensor(out=eqv[:], in0=vfs_t[:, :, 3:6],
                                        in1=vfs_t[:, :, 6:9], op=Alu.is_equal)
                alleq = ep.tile([P, EWoC], f32, tag="tf3")
                nc.vector.tensor_reduce(out=alleq[:], in_=eqv[:], axis=Ax.X,
                                        op=Alu.min)
                ovr = ep.tile([P, EWC], f32, tag="tf4")
                nc.vector.tensor_tensor(out=ovr[:], in0=samef_f[:],
                                        in1=alleq[:, 1:EWC + 1], op=Alu.mult)
                edge_state.update(ovr=ovr)

            def edge_chunk3():
                # face normal cross products (e1 x e2)
                e1t = ep.tile([P, EWoC, 3], f32, tag="e1")
                nc.vector.tensor_tensor(out=e1t[:], in0=vfs_t[:, :, 3:6],
                                        in1=vfs_t[:, :, 0:3], op=Alu.subtract)
                e2t = ep.tile([P, EWoC, 3], f32, tag="e2")
                nc.vector.tensor_tensor(out=e2t[:], in0=vfs_t[:, :, 6:9],
                                        in1=vfs_t[:, :, 0:3], op=Alu.subtract)
                n3 = ep.tile([P, EWoC, 3], f32, tag="n3")
                for k in range(3):
                    ka, kb = (k + 1) % 3, (k + 2) % 3
                    m1 = ep.tile([P, EWoC], f32, tag="tm1")
                    m2 = ep.tile([P, EWoC], f32, tag="tm2")
                    nc.vector.tensor_tensor(out=m1[:], in0=e1t[:, :, ka],
                                            in1=e2t[:, :, kb], op=Alu.mult)
                    nc.vector.tensor_tensor(out=m2[:], in0=e1t[:, :, kb],
                                            in1=e2t[:, :, ka], op=Alu.mult)
                    nc.vector.tensor_tensor(out=n3[:, :, k], in0=m1[:], in1=m2[:],
                                            op=Alu.subtract)
                edge_state.update(n3=n3)

            def edge_chunk4():
                n3 = edge_state["n3"]
                nsq = ep.tile([P, EWoC], f32, tag="tm3")
                nc.vector.tensor_tensor(out=nsq[:], in0=n3[:, :, 0],
                                        in1=n3[:, :, 0], op=Alu.mult)
                for k in (1, 2):
                    mk = ep.tile([P, EWoC], f32, tag="tm1")
                    nc.vector.tensor_tensor(out=mk[:], in0=n3[:, :, k],
                                            in1=n3[:, :, k], op=Alu.mult)
                    nc.vector.tensor_tensor(out=nsq[:], in0=nsq[:], in1=mk[:],
                                            op=Alu.add)
                nc.scalar.activation(nsq[:], nsq[:], Act.Sqrt)
                nc.vector.tensor_scalar(out=nsq[:], in0=nsq[:], scalar1=EPS_NRM,
                                        scalar2=None, op0=Alu.max)
                nc.vector.reciprocal(nsq[:], nsq[:])
                for k in range(3):
                    nc.vector.tensor_tensor(out=n3[:, :, k], in0=n3[:, :, k],
                                            in1=nsq[:], op=Alu.mult)

            def edge_chunk5():
                n3 = edge_state["n3"]
                p2f = edge_state["p2f"]
                ovr = edge_state["ovr"]
                totali = edge_state["totali"]
                violi = edge_state["violi"]
                prod = ep.tile([P, EWC, 3], f32, tag="e1")
                nc.vector.tensor_tensor(out=prod[:], in0=n3[:, 1:EWC + 1, :],
                                        in1=n3[:, 2:EWC + 2, :], op=Alu.mult)
                cosa = ep.tile([P, EWC], f32, tag="tf1")
                nc.vector.tensor_reduce(out=cosa[:], in_=prod[:], axis=Ax.X,
                                        op=Alu.add)
                nc.scalar.activation(cosa[:], cosa[:], Act.Relu, bias=nbias[:, :1])
                d5 = ep.tile([P, EWC], f32, tag="tf3")
                nc.scalar.activation(d5[:], cosa[:], Act.Copy, bias=0.5,
                                     scale=-1.0)
                nc.vector.tensor_tensor(out=d5[:], in0=d5[:], in1=ovr[:],
                                        op=Alu.mult)
                nc.vector.tensor_tensor(out=cosa[:], in0=cosa[:], in1=d5[:],
                                        op=Alu.add)
                nc.vector.tensor_tensor(out=cosa[:], in0=cosa[:], in1=p2f[:],
                                        op=Alu.mult)
                spart = ep.tile([P, 1], f32, tag="s3")
                nc.vector.tensor_reduce(out=spart[:], in_=cosa[:], axis=Ax.X,
                                        op=Alu.add)
                cnt2p = ep.tile([P, 1], f32, tag="s4")
                nc.vector.tensor_reduce(out=cnt2p[:], in_=p2f[:], axis=Ax.X,
                                        op=Alu.add)
                epk = ep.tile([P, 4], f32, tag="s5")
                nc.vector.tensor_copy(epk[:, 0:1], totali[:])
                nc.vector.tensor_copy(epk[:, 1:2], cnt2p[:])
                nc.vector.tensor_copy(epk[:, 2:3], spart[:])
                nc.vector.tensor_copy(epk[:, 3:4], violi[:])
                nc.sync.dma_start(epart_o.ap(), epk[:])

            edge_chunks = [edge_chunk0, edge_chunk1, edge_chunk2, edge_chunk3,
                           edge_chunk4, edge_chunk5]

            # ---- chamfer ----
            colacc0 = accp.tile([P, M], f16)
            colacc1 = accp.tile([P, M], f16)
            nnidx_i = cpool.tile([P, IB], i32)
            rowmin_all = cpool.tile([P, IB], f32)

            for ib in range(IB):
                slab = sbig.tile([P, M], f16, tag="slab")
                for jq in range(4):
                    psq = pp.tile([P, 2048], f32, tag="psq")
                    for k in range(4):
                        j0 = (jq * 4 + k) * 512
                        nc.tensor.matmul(psq[:, k * 512:(k + 1) * 512],
                                         lhsT=p5_sb[:, ib * P:(ib + 1) * P],
                                         rhs=g5_sb[:, j0:j0 + 512],
                                         start=True, stop=True)
                    # drain PSUM quarter -> f16 slab (ACT engine, casts)
                    nc.scalar.activation(slab[:, jq * 2048:(jq + 1) * 2048],
                                         psq[:], Act.Copy)
                    # spill this quarter for the winning-tile gather; the
                    # gather only waits ~one quarter-transfer after staging
                    nc.sync.dma_start(
                        mirror.ap()[ib * P * T:(ib + 1) * P * T, :]
                        .rearrange("(p t) k -> p t k", p=P)[:, 8 * jq:8 * jq + 8],
                        slab[:, jq * 2048:(jq + 1) * 2048]
                        .rearrange("p (t k) -> p t k", t=8))

                # tile-local min-fold tree: [P, 16, 512] -> [P, 16, 8]
                s3 = slab[:].rearrange("p (t k) -> p t k", t=T)
                lv = s3
                w = 256
                for li in range(5):
                    nxt = trp.tile([P, T, w // 2], f16, tag=f"L{li}")
                    nc.vector.tensor_tensor(out=nxt[:], in0=lv[:, :, 0:w // 2],
                                            in1=lv[:, :, w // 2:w], op=Alu.min)
                    lv = nxt
                    w //= 2
                rn = rnp.tile([P, 1], f16, tag="rn")
                nc.vector.tensor_reduce(out=rn[:], in_=lv[:], axis=Ax.XY,
                                        op=Alu.min)
                nc.scalar.activation(rowmin_all[:, ib:ib + 1], rn[:], Act.Copy)

                # winning tile t* = first tile whose L6 entry equals rn
                # (cand entries: t*-64 at matches, 0 elsewhere; min = t*-64)
                cand16 = swork.tile([P, T * 8], f32, tag="cand16")
                nc.vector.scalar_tensor_tensor(
                    out=cand16[:], in0=lv[:].rearrange("p t k -> p (t k)"),
                    scalar=rn[:, :1], in1=iotaT8MB[:],
                    op0=Alu.is_equal, op1=Alu.mult)
                argtm = swork.tile([P, 1], f32, tag="argtm")  # t* - 64
                nc.vector.tensor_reduce(out=argtm[:], in_=cand16[:], axis=Ax.X,
                                        op=Alu.min)
                # mirror row = ib*P*T + p*T + (argtm + 64)
                ridx_f = swork.tile([P, 1], f32, tag="ridx_f")
                nc.vector.scalar_tensor_tensor(out=ridx_f[:], in0=argtm[:],
                                               scalar=float(ib * P * T + 64),
                                               in1=rowb_f[:], op0=Alu.add,
                                               op1=Alu.add)
                ridx_i = swork.tile([P, 1], i32, tag="ridx_i")
                nc.vector.tensor_copy(ridx_i[:], ridx_f[:])
                win = swork.tile([P, 256], f16, tag="win")
                nc.gpsimd.indirect_dma_start(
                    out=win[:], out_offset=None, in_=mirror.ap(),
                    in_offset=bass.IndirectOffsetOnAxis(ap=ridx_i[:, :1], axis=0))

                # column-min accumulate (f16, DVE 2x; copies are 4x);
                # two accumulators: colacc0 streams out mid-loop on the ACT
                # queue, colacc1's final merge is split so its DMA overlaps
                acc = colacc0 if ib < 4 else colacc1
                if ib % 4 == 0:
                    nc.vector.tensor_copy(acc[:], slab[:])
                elif ib == 3:
                    for h in range(2):
                        sl = slice(h * (M // 2), (h + 1) * (M // 2))
                        nc.vector.tensor_tensor(out=acc[:, sl], in0=acc[:, sl],
                                                in1=slab[:, sl], op=Alu.min)
                        nc.sync.dma_start(colacc0_o.ap()[:, sl], acc[:, sl])
                elif ib == IB - 1:
                    for h in range(2):
                        sl = slice(h * (M // 2), (h + 1) * (M // 2))
                        nc.vector.tensor_tensor(out=acc[:, sl], in0=acc[:, sl],
                                                in1=slab[:, sl], op=Alu.min)
                        nc.sync.dma_start(colacc1_o.ap()[:, sl], acc[:, sl])
                else:
                    nc.vector.tensor_tensor(out=acc[:], in0=acc[:],
                                            in1=slab[:], op=Alu.min)

                for ci in (2 * ib, 2 * ib + 1):
                    if ci < len(edge_chunks):
                        edge_chunks[ci]()
                # u*+1 = sum over the window of (win == rn) * (iota+1)
                cand = swork.tile([P, 256], f32, tag="cand")
                us = swork.tile([P, 1], f32, tag="us")
                nc.vector.scalar_tensor_tensor(out=cand[:], in0=win[:],
                                               scalar=rn[:, :1],
                                               in1=iota256p1[:],
                                               op0=Alu.is_equal, op1=Alu.mult,
                                               accum_out=us[:])
                # j+1-16384 = (t*-64)*256 + (u*+1); j -> int (host clips)
                jp1 = swork.tile([P, 1], f32, tag="jp1")
                nc.vector.scalar_tensor_tensor(out=jp1[:], in0=argtm[:],
                                               scalar=256.0, in1=us[:],
                                               op0=Alu.mult, op1=Alu.add)
                nc.vector.tensor_scalar(out=nnidx_i[:, ib:ib + 1],
                                        in0=jp1[:], scalar1=16383.0,
                                        scalar2=None, op0=Alu.add)

            nc.sync.dma_start(nnidx_o.ap(), nnidx_i[:])
            nc.sync.dma_start(rowmin_o.ap(), rowmin_all[:])

    nc.compile()
    return nc


def _edge_host_inputs(verts, faces):
    """Host provides ORDERING + gathered layout only (lexsort + indexing);
    the device verifies sortedness and does all the arithmetic."""
    a = faces.reshape(-1).astype(np.int32)
    b = np.roll(faces, -1, axis=1).reshape(-1).astype(np.int32)
    lo = np.minimum(a, b)
    hi = np.maximum(a, b)
    perm = np.lexsort((hi, lo)).astype(np.int32)   # stable key order

    loS = np.full(TEP, 20001, np.int64)
    hiS = np.zeros(TEP, np.int64)
    eidS = np.zeros(TEP, np.int32)
    loS[:TE] = lo[perm]
    hiS[:TE] = hi[perm]
    eidS[:TE] = perm
    loS = loS.astype(np.int32)
    hiS = hiS.astype(np.int32)
    vfS = np.zeros((TEP, 9), np.float32)
    vfS[:TE] = verts[faces[perm // 3]].reshape(TE, 9)

    def overlap(arr, lo_sent, hi_sent):
        out = np.empty((P, EWo) + arr.shape[1:], arr.dtype)
        for c in range(EWo):
            i = np.arange(P) * EW + c - 1
            valid = (i >= 0) & (i < TEP)
            out[valid, c] = arr[i[valid]]
            out[~valid, c] = lo_sent if (c == 0) else hi_sent
        return out

    return {
        "elo": overlap(loS, -1, -2),
        "ehi": overlap(hiS, -1, -2),
        "eid": overlap(eidS, 0, 0),
        "vfs": overlap(vfS, 0.0, 0.0),
    }


def _lift_p(pts):
    """[K,3] -> [5,K] rows (x, y, z, |p|^2, 1)."""
    k = pts.shape[0]
    out = np.empty((5, k), np.float32)
    out[0:3] = pts.T
    out[3] = (pts * pts).sum(-1)
    out[4] = 1.0
    return out


def _lift_g(pts):
    """[M,3] -> [5,M] rows (-2x, -2y, -2z, 1, |g|^2)."""
    m = pts.shape[0]
    out = np.empty((5, m), np.float32)
    out[0:3] = -2.0 * pts.T
    out[3] = 1.0
    out[4] = (pts * pts).sum(-1)
    return out


def kernel(pred_sdf, gt_sdf, extracted_vertices, extracted_faces, gt_vertices,
           gt_faces, pred_points, gt_points, pred_normals, gt_normals):
    global _CACHED_NC
    if _CACHED_NC is None:
        _CACHED_NC = _build_program()
    nc = _CACHED_NC

    pp_full = np.asarray(pred_points, np.float32)[0]     # [N,3]
    gp_full = np.asarray(gt_points, np.float32)[0]       # [M,3]
    pn_full = np.asarray(pred_normals, np.float32)[0]
    gn_full = np.asarray(gt_normals, np.float32)[0]
    ps_full = np.asarray(pred_sdf, np.float32).reshape(-1)
    gs_full = np.asarray(gt_sdf, np.float32).reshape(-1)

    g5 = _lift_g(gp_full)
    edge_in = _edge_host_inputs(np.asarray(extracted_vertices, np.float32),
                                np.asarray(extracted_faces))
    in_maps = []
    for c in range(NC_CORES):
        rows = pp_full[c * NPC:(c + 1) * NPC]
        # column order (ib, p): column ib*128+p <-> core row p*8+ib
        p5c = _lift_p(rows)                               # [5, NPC] core-row order
        p5c = p5c.reshape(5, P, IB).transpose(0, 2, 1).reshape(5, NPC).copy()
        in_maps.append({
            "p5": p5c,
            "g5": g5,
            "ps": ps_full[c * NSC:(c + 1) * NSC].reshape(P, NSC // P).copy(),
            "gs": gs_full[c * NSC:(c + 1) * NSC].reshape(P, NSC // P).copy(),
            # per-core column shard of the sorted edge layout
            **{k: np.ascontiguousarray(v[:, c * EWC:c * EWC + EWoC])
               for k, v in edge_in.items()},
        })

    res = run_bass_kernel_spmd(nc, in_maps, core_ids=list(range(NC_CORES)),
                               trace=KERNEL_TRACE)
    if KERNEL_TRACE and res.exec_time_ns is not None:
        print(f"HW exec time: {res.exec_time_ns} ns")
    if TRACE_SINK is not None and res.instructions_and_trace is not None:
        TRACE_SINK["insts"] = res.instructions_and_trace[0]

    # ---- host combine ----
    rowmin_sum = 0.0
    sdf_sum = 0.0
    colmin = np.full(M, np.inf, np.float64)
    nn_global = np.empty(N, np.int64)
    for c in range(NC_CORES):
        r = res.results[c]
        rowmin_sum += r["rowmin"].astype(np.float64).sum()
        sdf_sum += r["sdfsum"].astype(np.float64).sum()
        # device row (p, ib) = pred point c*NPC + p*IB + ib
        nn_global[c * NPC:(c + 1) * NPC] = (
            r["nnidx"].astype(np.int64).reshape(-1))
        for key in ("colacc0", "colacc1"):
            ca = np.asarray(r[key])
            if ca.dtype != np.float16:
                ca = ca.view(np.float16) if ca.dtype.itemsize == 2 else ca
            colmin = np.minimum(colmin, ca.astype(np.float64).min(axis=0))

    sdf_l = SDF_W * sdf_sum / NS
    min_p2g = rowmin_sum / N
    min_g2p = colmin.mean()
    chamfer_l = CHAMFER_W * (min_p2g + min_g2p)
    # normal consistency from device nnidx (cosine math on host)
    matched = gn_full[np.clip(nn_global, 0, M - 1)]
    pnn = np.maximum(np.linalg.norm(pn_full, axis=-1), EPS_COS)
    gnn = np.maximum(np.linalg.norm(matched, axis=-1), EPS_COS)
    cosv = (pn_full * matched).sum(-1) / (pnn * gnn)
    normal_l = NORMAL_W * (1.0 - np.abs(cosv)).mean()

    ep = sum(res.results[c]["epart"].astype(np.float64)
             for c in range(NC_CORES))
    viol = ep[:, 3].sum()
    if viol != 0:
        raise RuntimeError(f"device sort-order verification failed: {viol}")
    total = ep[:, 0].sum() - 1.0      # minus the padding run
    cnt2 = ep[:, 1].sum()
    s2 = ep[:, 2].sum()
    edge = s2 / max(cnt2, 1.0) if cnt2 > 0 else 0.0
    bad = total - cnt2
    wt = bad / max(total, 1.0) if total > 0 else 0.0
    edge_l = EDGE_W * float(edge)
    wt_l = WATERTIGHT_W * float(wt)

    total = sdf_l + chamfer_l + normal_l + edge_l + wt_l
    return (np.float32(sdf_l), np.float32(chamfer_l), np.float32(normal_l),
            np.float32(edge_l), np.float32(wt_l), np.float32(total))
